# revision 26
# baseline (speedup 1.0000x reference)
"""Distributed attention kernel for 8 TRN2 NeuronCores.

Problem: B=2, T=2048, D=1024, H=16 heads, HD=64.
  q = x @ Wq.T + bq ; k = x @ Wk.T + bk ; v = q  (source quirk)
  S = q_h k_h^T / sqrt(D) ; P = softmax(S) ; o = P v_h ; concat heads.

Sharding: core c -> (batch b = c//4, head-group g = c%4, 4 heads each).
Each core is fully independent (no collectives).

Design notes (vs the v1 baseline at ~205us; this version ~176us):
  - The kernel is ScalarE-bound: exp over the scores is 16.8M elements
    per core at 1 elem/cycle/lane @1.2GHz ~= 132us of ACT busy.  v1
    wasted ~35us of startup, ~32us of mid-kernel ACT idle (PE-transpose
    phases for v + head-pair boundaries) and re-throttled HAM twice.
  - PSUM layout is the key constraint: S^T exp groups are one jt-pair
    (FD=1024, both heads side by side -> every S^T matmul runs as a
    row-packed concurrent pair), double-buffered = 4 banks; po
    accumulators = 2 banks; the remaining 2 banks are a dedicated
    prework tag ("w") so projections and v-transposes overlap the
    attention pipeline without stealing its psum slots (that slot
    contention is what serialized v1/v3).
  - v (=q in [key, dim] layout): one [128,128] PE transpose per
    (head-pair, key tile) gives both heads' v tiles at once; a single
    strided DVE copy evicts both.  The v region is memset to 1.0 so
    col 64 of each 128-col slot is the ones-column that folds the
    softmax denominator into PV.
  - Loads are split by first consumer (w quarter for hp0-q, then x
    tb0, then hp0-k quarter, ...) and interleaved w/x across the two
    DMA queues so the first projection chain starts as tiles arrive.
  - Issue order is a flattened software pipeline over all 128
    (hp, ib, jt) groups at depth 2: S^T+exp of step i issues with the
    PV of step i-2, so two queued PVs of slack sit behind every
    boundary's critical S' -> ACT' chain and ScalarE never drains at
    ib/hp seams; prework drips in <=0.9us fragments at steps chosen
    to avoid pipeline fill points, with hp1's tail pieces placed
    inside hp1-ib0 where PE idles.
  - DVE exp offload was tried and rejected: a bf16/f16 squaring chain
    is numerically too coarse near 1.0, and pushing VectorE hard put
    the chip into P0 power-throttle (everything -20%); DVE_EXP_SLOTS
    remains as a hook but must stay empty.
"""

import os
import numpy as np
import ml_dtypes

import concourse.bass as bass
import concourse.tile as tile
from concourse import bacc, mybir
from concourse.bass_utils import run_bass_kernel_spmd

B, T, D, H = 2, 2048, 1024, 16
HD = 64
NCORES = 8
HPC = 4          # heads per core
JG = HPC * HD    # 256 output dims per core
KT = 8           # contraction tiles of 128 over D
IB = 512         # query block
NIB = T // IB    # 4
NJT = T // 128   # 16 key tiles of 128
BF16 = mybir.dt.bfloat16
F32 = mybir.dt.float32

# (slot_idx 0..7, jt 0..15) exp groups handled by VectorE instead of
# ScalarE (slot_idx = hp*NIB + ib).  hp1's query blocks (slots 4-7) have
# the most DVE slack (their prework ran under hp0); hp0's ib1..3 carry
# hp1's prework copies so get fewer.
DVE_EXP_SLOTS: set = set()


def build_nc():
    nc = bacc.Bacc(None, target_bir_lowering=False, debug=False)

    xT = nc.declare_dram_parameter("xT", [1024, T], BF16, isOutput=False)
    wT = nc.declare_dram_parameter("wT", [1024, 2 * JG], BF16, isOutput=False)
    bias = nc.declare_dram_parameter("bias", [128, 4], F32, isOutput=False)
    idn = nc.declare_dram_parameter("idn", [128, 128], BF16, isOutput=False)
    out = nc.declare_dram_parameter("out", [JG + HPC, T], F32, isOutput=True)

    with tile.TileContext(nc) as tc:
        with (
            tc.tile_pool(name="const", bufs=1) as const_pool,
            tc.tile_pool(name="xw", bufs=1) as xw_pool,
            tc.tile_pool(name="qk", bufs=1) as qk_pool,
            tc.tile_pool(name="v", bufs=1) as v_pool,
            tc.tile_pool(name="p", bufs=6) as p_pool,
            tc.tile_pool(name="ev", bufs=4) as ev_pool,
            tc.tile_pool(name="psS", bufs=2, space="PSUM") as psS,
            tc.tile_pool(name="psW", bufs=2, space="PSUM") as psW,
            tc.tile_pool(name="psO", bufs=2, space="PSUM") as psO,
        ):
            # v storage: 64 slots (4 heads x 16 key tiles) of 128 cols each;
            # cols 0..63 = v data, col 64 = ones (preset by the memset).
            v_big = v_pool.tile([128, 128 * 4 * NJT], BF16, tag="vbig",
                                name="v_big")
            nc.vector.memset(v_big[:, :], 1.0)

            # Startup-critical loads (first proj chain needs all 8 w tiles +
            # all 8 x tb0 tiles): interleave across both DMA queues first;
            # ident/bias follow; x tb1..3 stream behind.
            wt = [xw_pool.tile([128, 2 * JG], BF16, tag=f"w{k}", name=f"w{k}")
                  for k in range(KT)]
            xt = [xw_pool.tile([128, T], BF16, tag=f"x{k}", name=f"x{k}")
                  for k in range(KT)]
            # w loads split by consumer so the first proj chain (hp0 q,
            # cols 0:128) is gated by the minimum bytes, then hp0 k;
            # hp1's halves ride behind the x stream.
            for k in range(KT):
                eng = nc.sync if k % 2 == 0 else nc.gpsimd
                eng.dma_start(wt[k][:, 0:128], wT[k * 128:(k + 1) * 128, 0:128])
                eng.dma_start(xt[k][:, 0:IB], xT[k * 128:(k + 1) * 128, 0:IB])
            for k in range(KT):
                eng = nc.sync if k % 2 == 0 else nc.gpsimd
                eng.dma_start(wt[k][:, 2 * JG // 2:2 * JG // 2 + 128],
                              wT[k * 128:(k + 1) * 128,
                                 2 * JG // 2:2 * JG // 2 + 128])
            ident = const_pool.tile([128, 128], BF16, tag="ident", name="ident")
            nc.sync.dma_start(ident[:, :], idn[:, :])
            bias_sb = const_pool.tile([128, 4], F32, tag="bias", name="bias_sb")
            nc.sync.dma_start(bias_sb[:, :], bias[:, :])
            for tb in range(1, NIB):
                cs = slice(tb * IB, (tb + 1) * IB)
                for k in range(KT):
                    eng = nc.sync if k % 2 == 0 else nc.gpsimd
                    eng.dma_start(xt[k][:, cs], xT[k * 128:(k + 1) * 128, cs])
            for k in range(KT):
                eng = nc.sync if k % 2 == 0 else nc.gpsimd
                eng.dma_start(wt[k][:, 128:256], wT[k * 128:(k + 1) * 128, 128:256])
                eng.dma_start(wt[k][:, 384:512], wT[k * 128:(k + 1) * 128, 384:512])

            qT = [qk_pool.tile([128, T], BF16, tag=f"qT{j}", name=f"qT{j}")
                  for j in range(2)]
            kTt = [qk_pool.tile([128, T], BF16, tag=f"kT{j}", name=f"kT{j}")
                   for j in range(2)]

            # ---- prework pieces (dedicated "w" psum slots).  Proj is two
            # 4-matmul fragments so a dripped piece never puts more than
            # ~0.9us of PE work ahead of the attention critical path.
            def proj_pieces(hp, tb, w_idx):
                dst = qT if w_idx == 0 else kTt
                hold = {}

                def frag_a():
                    hold["ps"] = psW.tile([128, IB], F32, tag="w",
                                          name="ps_proj")
                    for k in range(KT // 2):
                        nc.tensor.matmul(
                            hold["ps"][:, :],
                            wt[k][:, w_idx * JG + hp * 128:
                                  w_idx * JG + (hp + 1) * 128],
                            xt[k][:, tb * IB:(tb + 1) * IB],
                            start=(k == 0), stop=False,
                        )

                def frag_b():
                    for k in range(KT // 2, KT):
                        nc.tensor.matmul(
                            hold["ps"][:, :],
                            wt[k][:, w_idx * JG + hp * 128:
                                  w_idx * JG + (hp + 1) * 128],
                            xt[k][:, tb * IB:(tb + 1) * IB],
                            start=False, stop=(k == KT - 1),
                        )
                    nc.vector.tensor_scalar(
                        dst[hp][:, tb * IB:(tb + 1) * IB], hold["ps"][:, :],
                        bias_sb[:, w_idx * 2 + hp:w_idx * 2 + hp + 1],
                        None, mybir.AluOpType.add)

                return [frag_a, frag_b]

            def trans_piece(hp, jt):
                def frag():
                    # one [128,128] PE transpose yields both heads' v tiles
                    pt = psW.tile([128, 128], BF16, tag="w", name="pt_tr")
                    nc.tensor.transpose(
                        pt[:, :], qT[hp][:, jt * 128:(jt + 1) * 128],
                        ident[:, :])
                    for hh in range(2):
                        vi = (hp * 2 + hh) * NJT + jt
                        nc.vector.tensor_copy(
                            v_big[:, vi * 128:vi * 128 + 64],
                            pt[:, hh * 64:(hh + 1) * 64])
                return frag

            def hp_pieces(hp, from_tb=0):
                ps = []
                for tb in range(from_tb, NIB):
                    ps += proj_pieces(hp, tb, 0)
                    ps += proj_pieces(hp, tb, 1)
                    ps += [trans_piece(hp, jt)
                           for jt in range(4 * tb, 4 * tb + 4)]
                return ps

            # ---- attention pieces, software-pipelined at issue time:
            # S^T+exp for group g+1 is issued before PV of group g so the
            # scheduler keeps ScalarE fed across group/ib/hp boundaries.
            def s_exp(hp, ib, jt):
                slot_idx = hp * NIB + ib
                ps = psS.tile([128, 2 * IB], F32, tag="s", name="ps_s")
                for hh in range(2):
                    off = 64 * hh
                    nc.tensor.matmul(
                        ps[:, hh * IB:(hh + 1) * IB],
                        kTt[hp][off:off + 64, jt * 128:(jt + 1) * 128],
                        qT[hp][off:off + 64, ib * IB:(ib + 1) * IB],
                        start=True, stop=True,
                        tile_position=(off, 0),
                    )
                pexp = p_pool.tile([128, 2 * IB], BF16, tag="p", name="pexp")
                nc.scalar.activation(
                    pexp[:, :], ps[:, :],
                    mybir.ActivationFunctionType.Exp,
                    scale=1.0 / 32.0,
                )
                return pexp

            def pv(hp, ib, jt, po, pexp):
                for hh in range(2):
                    vi = (hp * 2 + hh) * NJT + jt
                    nc.tensor.matmul(
                        po[hh][:, :],
                        v_big[:, vi * 128:vi * 128 + 65],
                        pexp[:, hh * IB:(hh + 1) * IB],
                        start=(jt == 0), stop=(jt == NJT - 1),
                    )

            def evict(hp, ib, po):
                for hh in range(2):
                    h = 2 * hp + hh
                    ev = ev_pool.tile([65, IB], F32, tag="ev", name="ev")
                    nc.vector.tensor_copy(ev[:, :], po[hh][:, :])
                    eng = nc.gpsimd if hh == 0 else nc.sync
                    eng.dma_start(
                        out[h * 65:h * 65 + 65, ib * IB:(ib + 1) * IB],
                        ev[:, :])

            # ---- issue order: flattened (hp, ib, jt) pipeline with the
            # S^T+exp of step i+1 issued before the PV of step i, and
            # prework pieces dripped at steps chosen to avoid pipeline
            # fill points (ib starts).  hp0: tb0 proj up front, the rest
            # 2 pieces per ib0-step (deadline-driven).  hp1: 23 pieces
            # across hp0's ib1..3 (even steps), the last 9 inside
            # hp1-ib0 where PE otherwise idles.
            p0q = proj_pieces(0, 0, 0)
            p0k = proj_pieces(0, 0, 1)
            for f in (p0q[0], p0k[0], p0q[1], p0k[1]):
                f()
            drip = {}
            q0 = [trans_piece(0, 0), trans_piece(0, 1),
                  trans_piece(0, 2), trans_piece(0, 3)] + hp_pieces(0, 1)
            for st in range(14):
                drip[st] = q0[2 * st:2 * st + 2]
            q1 = hp_pieces(1)
            slots1 = [s for s in range(18, 63, 2)] + list(range(65, 75))
            for st, f in zip(slots1, q1):
                drip.setdefault(st, []).append(f)

            flat = [(hp, ib, jt)
                    for hp in range(2) for ib in range(NIB)
                    for jt in range(NJT)]
            po_map = {}
            pend = []

            def pop_pv():
                php, pib, pjt, ppexp = pend.pop(0)
                if pjt == 0:
                    po_map[(php, pib)] = [
                        psO.tile([65, IB], F32, tag="o", name=f"po{hh}")
                        for hh in range(2)]
                po = po_map[(php, pib)]
                pv(php, pib, pjt, po, ppexp)
                if pjt == NJT - 1:
                    evict(php, pib, po)
                    del po_map[(php, pib)]

            for st, (hp, ib, jt) in enumerate(flat):
                pexp = s_exp(hp, ib, jt)
                for u in drip.get(st, ()):
                    u()
                pend.append((hp, ib, jt, pexp))
                if len(pend) > 2:
                    pop_pv()
            while pend:
                pop_pv()
    nc.finalize()
    return nc


_NC_CACHE = None


def _ensure_ntff_hook():
    """Provide the antenv.axon_hooks NTFF-profiling shim this image lacks."""
    import sys
    import types
    import ctypes
    import contextlib

    if "antenv.axon_hooks" in sys.modules:
        return
    mod = types.ModuleType("antenv.axon_hooks")
    state = {"hook": None}
    mod.set_axon_ntff_profile_hook = lambda h: state.__setitem__("hook", h)
    mod.get_axon_ntff_profile_hook = lambda: state["hook"]
    sys.modules["antenv.axon_hooks"] = mod
    try:
        import antenv
        antenv.axon_hooks = mod
    except ImportError:
        pass
    so = "/opt/axon/libaxon_pjrt.so"
    if not os.path.exists(so):
        return
    lib = ctypes.CDLL(so)
    if not hasattr(lib, "axon_start_nrt_profile"):
        return
    lib.axon_start_nrt_profile.argtypes = [
        ctypes.POINTER(ctypes.c_int64), ctypes.c_size_t]
    lib.axon_start_nrt_profile.restype = ctypes.c_int64
    lib.axon_stop_nrt_profile.argtypes = [ctypes.c_char_p]
    lib.axon_stop_nrt_profile.restype = ctypes.c_int64

    @contextlib.contextmanager
    def _hook(output_dir, device_ids):
        import jax
        jax.devices()
        if device_ids:
            ids = (ctypes.c_int64 * len(device_ids))(*device_ids)
            rc = lib.axon_start_nrt_profile(ids, len(device_ids))
        else:
            rc = lib.axon_start_nrt_profile(None, 0)
        if rc != 0:
            raise RuntimeError(f"axon_start_nrt_profile rc={rc}")
        try:
            yield
        finally:
            n = lib.axon_stop_nrt_profile(str(output_dir).encode())
            print(f"ntff profile: {n} file(s) -> {output_dir}")

    mod.set_axon_ntff_profile_hook(_hook)


def kernel(x, Wq, bq, Wk, bk):
    global _NC_CACHE
    x = np.asarray(x, dtype=np.float32)
    Wq = np.asarray(Wq, dtype=np.float32)
    bq = np.asarray(bq, dtype=np.float32)
    Wk = np.asarray(Wk, dtype=np.float32)
    bk = np.asarray(bk, dtype=np.float32)

    bf = ml_dtypes.bfloat16
    in_maps = []
    for c in range(NCORES):
        b, g = c // 4, c % 4
        sl = slice(g * JG, (g + 1) * JG)
        w_all = np.concatenate([Wq[sl].T, Wk[sl].T], axis=1)  # [1024, 512]
        bias_all = np.stack(
            [bq[sl][0:128], bq[sl][128:256],
             bk[sl][0:128], bk[sl][128:256]], axis=1)  # [128, 4]
        in_maps.append({
            "xT": np.ascontiguousarray(x[b].T).astype(bf),
            "wT": w_all.astype(bf),
            "bias": bias_all.astype(np.float32),
            "idn": np.eye(128, dtype=np.float32).astype(bf),
        })

    if _NC_CACHE is None:
        _NC_CACHE = build_nc()
    nc = _NC_CACHE

    if int(os.environ.get("KERNEL_TRACE", "0")):
        _ensure_ntff_hook()
    res = run_bass_kernel_spmd(
        nc, in_maps, core_ids=list(range(NCORES)),
        trace=bool(int(os.environ.get("KERNEL_TRACE", "0"))),
        tmpdir=os.environ.get("KERNEL_TMPDIR") or None,
    )
    if res.exec_time_ns is not None:
        print(f"HW exec time: {res.exec_time_ns} ns")

    full = np.empty((B, T, D), np.float32)
    for c in range(NCORES):
        b, g = c // 4, c % 4
        oc = res.results[c]["out"].reshape(HPC, HD + 1, T)  # [4, 65, 2048]
        o = oc[:, 0:HD]                       # [4, 64, 2048]
        s = oc[:, HD:HD + 1]                  # [4, 1, 2048]
        blk = (o / s).transpose(2, 0, 1).reshape(T, JG)
        full[b, :, g * JG:(g + 1) * JG] = blk
    return full
